# revision 19
# baseline (speedup 1.0000x reference)
"""GAT 2-layer kernel for Trainium2, 8 NeuronCores — v4.

Design (from microbenchmark findings):
- The per-edge row gather (dma_gather, 256B rows, 1024 idx/instr, 4 SWDGE
  queues) runs at ~2.35ns/edge when the queues are continuously fed and
  nothing else competes for the DMA engines. The v3 kernel lost ~3x to
  contention (indirect a_dst DMAs at ~300ns/descriptor, shallow buffering,
  broadcast DMA traffic mid-phase).
- Per-core row permutation puts each core's OWN dst nodes at rows 0..6271,
  so per-tile a_dst vectors are extracted at compile-time-known positions
  during the build phase (no indirect DMA, no adsti input).
- Pad slots gather a POISON row whose a_src columns hold -60 =>
  exp(leakyrelu(-60+a_dst)) ~ e-12: the mask multiply is eliminated.
  Poison rows: 32767 (lo region) and 50175 (hi region) are reserved
  non-node rows via the permutation.
- T is built into two DRAM tiles (lo rows < 32768, hi rows >= 32768) so
  lo-region gathers start while the hi half is still being built.
- Edge math: one add (a_src+a_dst), two-op leakyrelu, ACT exp in place,
  and ONE broadcast-view multiply for all 8 heads.
"""

import numpy as np
import ml_dtypes
from contextlib import ExitStack

import concourse.bass as bass
import concourse.tile as tile
from concourse import bacc, mybir
from concourse.bass import ts, ds
from concourse.bass_utils import run_bass_kernel_spmd

BF16 = mybir.dt.bfloat16
F32 = mybir.dt.float32
I16 = mybir.dt.int16
NPBF16 = ml_dtypes.bfloat16

P = 128
NCORES = 8
N = 50000
E = 1600000
TPC = 49
NPAD = NCORES * TPC * P       # 50176
NOWN = TPC * P                # 6272 rows per core (own dst rows)
NREAL = N // NCORES           # 6250 real nodes per core
L = 4                         # edge slots per row
GSZ = 2                       # dst tiles per edge-phase group
BT = 8                        # node tiles per build iteration
NIDX = 1024                   # idx per dma_gather instruction
SPLIT = 32768                 # lo/hi region boundary (int16 idx range)
POISON_LO = SPLIT - 1         # row 32767: reserved, a_src = -60
POISON_HI = NPAD - 1          # row 50175: reserved, a_src = -60
NEG_SLOPE = 0.2
POISON_VAL = -60.0
TW = 128                      # table row width in bf16 (256B)

LAST_RESULTS = []


def _core_perms():
    """Per-core node->row permutations.

    Core k's own nodes (k*6250..(k+1)*6250) map to rows 0..6249 (rows
    6250..6271 junk-pad). Other nodes fill rows 6272.. skipping the two
    poison rows. Returns (node2row[k], row2node-ish via out slicing).
    """
    perms = []
    for k in range(NCORES):
        node2row = np.zeros(N, np.int64)
        own = np.arange(k * NREAL, (k + 1) * NREAL)
        node2row[own] = np.arange(NREAL)
        others = np.concatenate([np.arange(0, k * NREAL),
                                 np.arange((k + 1) * NREAL, N)])
        slots = np.setdiff1d(np.arange(NOWN, NPAD),
                             [POISON_LO, POISON_HI])[:len(others)]
        node2row[others] = slots
        perms.append(node2row)
    return perms


def _prep_core(src_r, dst_r, tpc=TPC):
    """Edge layout for ONE core. dst_r in [0, NOWN); src_r in [0, NPAD)."""
    g_tiles = tpc

    is_hi = (src_r >= SPLIT).astype(np.int64)
    order = np.lexsort((is_hi, dst_r))
    srcs = src_r[order]
    dsts = dst_r[order]
    is_hi = is_hi[order]

    deg_lo = np.bincount(dsts[is_hi == 0], minlength=NOWN)
    deg_hi = np.bincount(dsts[is_hi == 1], minlength=NOWN)
    rows_lo_n = (deg_lo + L - 1) // L
    rows_hi_n = (deg_hi + L - 1) // L

    def tile_chunks(rows_n):
        gro = np.concatenate([[0], np.cumsum(rows_n)])
        t0 = gro[np.arange(g_tiles) * P]
        rt = gro[np.minimum(np.arange(1, g_tiles + 1) * P, NOWN)] - t0
        ch = (rt + P - 1) // P
        return gro, t0, ch

    gro_lo, tstart_lo, RchL = tile_chunks(rows_lo_n)
    gro_hi, tstart_hi, RchH = tile_chunks(rows_hi_n)
    RchL = np.maximum(RchL, 1)

    return dict(srcs=srcs, dsts=dsts, is_hi=is_hi,
                deg_lo=deg_lo, deg_hi=deg_hi,
                rows_lo_n=rows_lo_n, rows_hi_n=rows_hi_n,
                gro_lo=gro_lo, tstart_lo=tstart_lo, RchL=RchL,
                gro_hi=gro_hi, tstart_hi=tstart_hi, RchH=RchH)


def _prep_edges(edge_index):
    perms = _core_perms()
    src = np.asarray(edge_index[0]).astype(np.int64)
    dst = np.asarray(edge_index[1]).astype(np.int64)
    owner = dst // NREAL

    cores = []
    for k in range(NCORES):
        m = owner == k
        cores.append(_prep_core(perms[k][src[m]], perms[k][dst[m]]))

    # shared shapes: max chunks per tile position across cores
    RchL = np.maximum.reduce([c["RchL"] for c in cores])
    RchH = np.maximum.reduce([c["RchH"] for c in cores])

    groups = []
    t0 = 0
    while t0 < TPC:
        groups.append((t0, min(GSZ, TPC - t0)))
        t0 += GSZ
    chunk_base = np.zeros((TPC, 2), np.int64)
    tile_chunk_lists = [[] for _ in range(TPC)]
    groups_meta = []
    cid = 0
    for (t0, gn) in groups:
        gbase = cid
        specs = []
        for ri, Rch in ((0, RchL), (1, RchH)):
            col0 = (cid - gbase) * L
            ncols = int(sum(Rch[t0 + j] for j in range(gn))) * L
            if ncols:
                specs.append((ri, col0, ncols, cid - gbase))
            for j in range(gn):
                chunk_base[t0 + j, ri] = cid
                for _ in range(int(Rch[t0 + j])):
                    tile_chunk_lists[t0 + j].append(cid)
                    cid += 1
        groups_meta.append((t0, gn, gbase, cid - gbase, specs))
    NRCH = cid

    rdf_arr = np.full((NCORES, P, NRCH), -1.0, np.float32)
    rdfR_arr = np.full((NCORES, NRCH * P), -1, np.int8)
    idxs_arr = np.zeros((NCORES, NRCH * L * P), np.int64)
    # poison defaults per chunk region
    for (t0, gn, gbase, RLG, specs) in groups_meta:
        for (ri, col0, ncols, chunk0) in specs:
            poison = POISON_LO if ri == 0 else POISON_HI - SPLIT
            s0 = (gbase + chunk0) * L * P
            idxs_arr[:, s0:s0 + ncols * P] = poison

    for k, c in enumerate(cores):
        for ri in (0, 1):
            rows_n = c["rows_lo_n"] if ri == 0 else c["rows_hi_n"]
            gro = c["gro_lo"] if ri == 0 else c["gro_hi"]
            tstart = c["tstart_lo"] if ri == 0 else c["tstart_hi"]
            tot = int(gro[-1])
            if tot == 0:
                continue
            row_node = np.repeat(np.arange(NOWN), rows_n)
            row_tile = row_node >> 7
            rit = np.arange(tot) - tstart[row_tile]
            r_cid = chunk_base[row_tile, ri] + rit // P
            r_p = rit % P
            rdf_arr[k, r_p, r_cid] = (row_node & 127).astype(np.float32)
            rdfR_arr[k, r_cid * P + r_p] = (row_node & 127).astype(np.int8)

        srcs, dsts, is_hi = c["srcs"], c["dsts"], c["is_hi"]
        e_t = dsts >> 7
        for ri in (0, 1):
            sel = is_hi == ri
            idx_e = np.where(sel)[0]
            if len(idx_e) == 0:
                continue
            d = dsts[idx_e]
            s = srcs[idx_e] - (0 if ri == 0 else SPLIT)
            gro = c["gro_lo"] if ri == 0 else c["gro_hi"]
            tstart = c["tstart_lo"] if ri == 0 else c["tstart_hi"]
            pos = np.arange(len(idx_e))
            seg0 = np.searchsorted(d, np.arange(NOWN), side="left")
            rank = pos - seg0[d]
            row_r = gro[d] + rank // L
            slot = rank % L
            rit = row_r - tstart[e_t[idx_e]]
            cidv = chunk_base[e_t[idx_e], ri] + rit // P
            pv = rit % P
            idxs_arr[k, (cidv * L + slot) * P + pv] = s

    TOT = NRCH * L * P
    assert TOT % 16 == 0
    idx16_arr = np.zeros((NCORES, P, TOT // 16), np.int16)
    for k in range(NCORES):
        w = idxs_arr[k].astype(np.int16).reshape(TOT // 16, 16).T
        for rep in range(8):
            idx16_arr[k, 16 * rep:16 * rep + 16, :] = w

    return dict(
        perms=perms,
        rdf=rdf_arr.astype(NPBF16), rdfR=rdfR_arr,
        idx16=idx16_arr,
        NRCH=NRCH, groups_meta=groups_meta,
        tile_chunk_lists=tile_chunk_lists,
        RLG_MAX=max(g[3] for g in groups_meta),
        RCHT_MAX=max(len(cl) for cl in tile_chunk_lists),
    )


def _build_layer_program(KIN, F_D, meta, layer):
    F_H = 64
    F_G = F_H + F_D          # aggregated row width (msg | w)
    MW = L * F_G
    KT = (KIN + P - 1) // P
    KP = min(KIN, P)
    NRCH = meta["NRCH"]
    RLG_MAX = meta["RLG_MAX"]
    RCHT_MAX = meta["RCHT_MAX"]
    g_tiles = NPAD // P                  # 392 node tiles (full table)
    lo_tiles = SPLIT // P                # 256
    n_bt = (g_tiles + BT - 1) // BT
    out_dt = BF16 if layer == 1 else F32
    eps = 1e-16
    CW = F_H // F_D

    nc = bacc.Bacc("TRN2", target_bir_lowering=False, debug=False,
                   num_devices=NCORES, num_swdge_queues=4,
                   dynamic_dma_scratch_size=32768)

    xT_in = nc.dram_tensor("xT", [KIN, NPAD], BF16, kind="ExternalInput").ap()
    wc_in = nc.dram_tensor("wc", [KIN, TW], BF16, kind="ExternalInput").ap()
    idx_in = nc.dram_tensor("idx16", [P, NRCH * L * P // 16], I16,
                            kind="ExternalInput").ap()
    rdf_in = nc.dram_tensor("rdf", [P, NRCH], BF16, kind="ExternalInput").ap()
    rdfR_in = nc.dram_tensor("rdfR", [P, NRCH * P], mybir.dt.int8,
                             kind="ExternalInput").ap()
    bias_in = nc.dram_tensor("bias", [1, F_H], F32, kind="ExternalInput").ap()
    pois_in = nc.dram_tensor("pois", [1, TW - F_H], BF16,
                             kind="ExternalInput").ap()
    out_dram = nc.dram_tensor("out", [TPC * P, F_H], out_dt,
                              kind="ExternalOutput").ap()

    with tile.TileContext(nc) as tc, ExitStack() as ctx:
        cpool = ctx.enter_context(tc.tile_pool(name="const", bufs=1))
        dpool = ctx.enter_context(tc.tile_pool(name="dram", bufs=1,
                                               space=bass.MemorySpace.DRAM))
        bpool = ctx.enter_context(tc.tile_pool(name="bld", bufs=3))
        epool = ctx.enter_context(tc.tile_pool(name="edge", bufs=2))
        gtpool = ctx.enter_context(tc.tile_pool(name="gt", bufs=5))
        opool = ctx.enter_context(tc.tile_pool(name="post", bufs=2))
        pps = ctx.enter_context(tc.tile_pool(name="psb", bufs=2,
                                             space=bass.MemorySpace.PSUM))
        ppe = ctx.enter_context(tc.tile_pool(name="pse", bufs=2,
                                             space=bass.MemorySpace.PSUM))
        pat = ctx.enter_context(tc.tile_pool(name="pat", bufs=2,
                                             space=bass.MemorySpace.PSUM))

        # ---- constants ----
        wc_sb = cpool.tile([KP, KT, TW], BF16)
        for kt in range(KT):
            nc.sync.dma_start(wc_sb[:, kt, :], wc_in[kt * KP:(kt + 1) * KP, :])
        bias_sb = cpool.tile([P, F_H], F32)
        nc.sync.dma_start(bias_sb[:], bias_in.to_broadcast((P, F_H)))
        pois_sb = cpool.tile([1, TW - F_H], BF16)
        nc.sync.dma_start(pois_sb[:], pois_in[:])
        iota_i = cpool.tile([P, 128], mybir.dt.int32)
        nc.gpsimd.iota(iota_i[:], pattern=[[1, 128]], channel_multiplier=0)
        iota_bf = cpool.tile([P, 1, 128], BF16)
        nc.vector.tensor_copy(iota_bf[:, 0, :], iota_i[:])
        iop_i = cpool.tile([P, 1], mybir.dt.int32)
        nc.gpsimd.iota(iop_i[:], pattern=[[0, 1]], channel_multiplier=1)
        iop8 = cpool.tile([P, 1, 1], mybir.dt.int8)
        nc.vector.tensor_copy(iop8[:, 0, :], iop_i[:])
        idx16_sb = cpool.tile([P, NRCH * L * P // 16], I16)
        nc.sync.dma_start(idx16_sb[:], idx_in[:])
        rdf_sb = cpool.tile([P, NRCH, 1], BF16)
        nc.sync.dma_start(rdf_sb[:, :, 0], rdf_in[:])
        adst_all = cpool.tile([P, TPC, F_D], BF16)

        # ---- phase 1: build T = [h | a_src | a_dst | pad] ----
        T_lo = dpool.tile([SPLIT, TW], BF16)
        T_hi = dpool.tile([NPAD - SPLIT, TW], BF16)
        for b in range(n_bt):
            bt = min(BT, g_tiles - b * BT)
            cols = bt * P
            xt = bpool.tile([KP, KT, BT, P], BF16)
            for kt in range(KT):
                nc.sync.dma_start(
                    xt[:, kt, 0:bt, :],
                    xT_in[kt * KP:(kt + 1) * KP, ds(b * BT * P, cols)])
            tcast = bpool.tile([P, BT, TW], BF16)
            if True:
                for j2 in range(0, bt, 2):
                    nj = min(2, bt - j2)
                    psB2 = pps.tile([P, 2, TW], F32)
                    for jj in range(nj):
                        for kt in range(KT):
                            nc.tensor.matmul(psB2[:, jj, :],
                                             xt[:, kt, j2 + jj, :],
                                             wc_sb[:, kt, :],
                                             start=(kt == 0),
                                             stop=(kt == KT - 1))
                    if (j2 // 2) % 2 == 0:
                        nc.vector.tensor_copy(tcast[:, j2:j2 + nj, :],
                                              psB2[:, 0:nj, :])
                    else:
                        nc.scalar.activation(
                            tcast[:, j2:j2 + nj, :], psB2[:, 0:nj, :],
                            mybir.ActivationFunctionType.Copy)
            else:
                for j in range(bt):
                    psB = pps.tile([P, TW], F32)
                    for kt in range(KT):
                        nc.tensor.matmul(psB[:], xt[:, kt, j, :],
                                         wc_sb[:, kt, :],
                                         start=(kt == 0),
                                         stop=(kt == KT - 1))
                    if j % 2 == 0:
                        nc.vector.tensor_copy(tcast[:, j, :], psB[:])
                    else:
                        nc.scalar.activation(
                            tcast[:, j, :], psB[:],
                            mybir.ActivationFunctionType.Copy)
            # own-tile a_dst extraction (tiles 0..TPC-1 are this core's)
            t_lo_g = b * BT
            if t_lo_g < TPC:
                nt = min(bt, TPC - t_lo_g)
                nc.vector.tensor_copy(
                    adst_all[:, t_lo_g:t_lo_g + nt, :],
                    tcast[:, 0:nt, F_H + F_D:F_H + 2 * F_D])
            tgt_row = b * BT * P
            WC = F_H + 8            # used table cols, 16B-aligned width
            if tgt_row < SPLIT:
                nc.scalar.dma_start(
                    T_lo[ds(tgt_row, cols), 0:WC].rearrange(
                        "(j p) c -> p j c", p=P),
                    tcast[:, 0:bt, 0:WC])
            else:
                nc.scalar.dma_start(
                    T_hi[ds(tgt_row - SPLIT, cols), 0:WC].rearrange(
                        "(j p) c -> p j c", p=P),
                    tcast[:, 0:bt, 0:WC])
        # poison rows: a_src (and a_dst) cols := -60
        nc.sync.dma_start(T_lo[POISON_LO:POISON_LO + 1, F_H:F_H + F_D],
                          pois_sb[:, 0:F_D])
        nc.sync.dma_start(T_hi[POISON_HI - SPLIT:POISON_HI - SPLIT + 1,
                               F_H:F_H + F_D], pois_sb[:, 0:F_D])

        # ---- phase 2: edge aggregation per group ----
        if layer == 2:
            zbuf = cpool.tile([P, TPC, F_H], F32)
            sums = cpool.tile([P, TPC], F32)

        tile_chunks = meta["tile_chunk_lists"]
        qrot = [0]

        def issue_gathers(Gt, gbase, specs, region=None):
            for (ri, col0, ncols, chunk0) in specs:
                if region is not None and ri != region:
                    continue
                nslots = ncols * P
                islot0 = (gbase + chunk0) * L * P
                tab = T_lo[:] if ri == 0 else T_hi[:]
                o = 0
                while o < nslots:
                    ni = min(NIDX, nslots - o)
                    i0 = (islot0 + o) // 16
                    nc.gpsimd.dma_gather(
                        out_ap=Gt[:, col0 + o // P:col0 + (o + ni) // P, :],
                        in_ap=tab,
                        idxs_ap=idx16_sb[:, i0:i0 + ni // 16],
                        num_idxs=ni, num_idxs_reg=ni, elem_size=TW,
                        queue_num=qrot[0] % 4)
                    qrot[0] += 1
                    o += ni

        HH = 5   # hoist first groups' lo gathers ahead of hi gathers
        pre_gt = {}
        gmeta = meta["groups_meta"]
        for gi in range(min(HH, len(gmeta))):
            (t0, gn, gbase, RLG, specs) = gmeta[gi]
            Gt = gtpool.tile([P, RLG_MAX * L, TW], BF16)
            pre_gt[gi] = Gt
            issue_gathers(Gt, gbase, specs, region=0)
        for gi in range(min(HH, len(gmeta))):
            (t0, gn, gbase, RLG, specs) = gmeta[gi]
            issue_gathers(pre_gt[gi], gbase, specs, region=1)

        for gi, (t0, gn, gbase, RLG, specs) in enumerate(gmeta):
            RLs = RLG * L
            if gi in pre_gt:
                Gt = pre_gt.pop(gi)
            else:
                Gt = gtpool.tile([P, RLG_MAX * L, TW], BF16)
                issue_gathers(Gt, gbase, specs)

            # S^T for a_dst expansion: S_T[d, c] = (rowdst[chunk, c] == d)
            rdfR_sb = epool.tile([P, RLG_MAX, 128], mybir.dt.int8)
            nc.sync.dma_start(
                rdfR_sb[:, 0:RLG, :].rearrange("p r d -> p (r d)"),
                rdfR_in[:, ds(gbase * P, RLG * P)])
            S_T = epool.tile([P, RLG_MAX, 128], BF16)
            nc.vector.tensor_tensor(
                S_T[:, 0:RLG, :],
                rdfR_sb[:, 0:RLG, :],
                iop8[:].to_broadcast((P, RLG, 128)),
                op=mybir.AluOpType.is_equal)
            St = epool.tile([P, RLG_MAX, 128], BF16)
            nc.vector.tensor_tensor(
                St[:, 0:RLG, :],
                rdf_sb[:, ds(gbase, RLG), :].to_broadcast((P, RLG, 128)),
                iota_bf[:].to_broadcast((P, RLG, 128)),
                op=mybir.AluOpType.is_equal)

            # a_dst expansion to rows (PE), per tile
            atsb = epool.tile([P, RLG_MAX, F_D], F32)
            for j in range(gn):
                t = t0 + j
                chunks = tile_chunks[t]
                psAt = pat.tile([P, RCHT_MAX * F_D], F32)
                for ci, cid in enumerate(chunks):
                    nc.tensor.matmul(
                        psAt[:, ci * F_D:(ci + 1) * F_D],
                        S_T[:, cid - gbase, :],
                        adst_all[:, t, :],
                        start=True, stop=True)
                # chunk list = lo run + hi run (each contiguous)
                runs, st_c = [], chunks[0]
                prev = st_c
                for cch in chunks[1:]:
                    if cch == prev + 1:
                        prev = cch
                        continue
                    runs.append((st_c, prev))
                    st_c = prev = cch
                runs.append((st_c, prev))
                for (a, bb) in runs:
                    nch = bb - a + 1
                    ci0 = chunks.index(a)
                    nc.scalar.activation(
                        atsb[:, a - gbase:a - gbase + nch, :],
                        psAt[:, ci0 * F_D:(ci0 + nch) * F_D].rearrange(
                            "p (c f) -> p c f", f=F_D),
                        mybir.ActivationFunctionType.Copy)

            # logits -> w = exp(leakyrelu(a_src + a_dst)), in place
            Lt = epool.tile([P, RLG_MAX * L, F_D], BF16)
            nc.vector.tensor_add(
                Lt[:, 0:RLs, :].rearrange("p (r l) f -> p r l f", l=L),
                Gt[:, 0:RLs, F_H:F_H + F_D].rearrange(
                    "p (r l) f -> p r l f", l=L),
                atsb[:, 0:RLG, :].unsqueeze(2).to_broadcast((P, RLG, L, F_D)))
            # exp(leakyrelu(x)) == max(exp(x), exp(slope*x)) exactly
            E2t = epool.tile([P, RLG_MAX * L, F_D], BF16)
            nc.scalar.activation(E2t[:, 0:RLs, :], Lt[:, 0:RLs, :],
                                 mybir.ActivationFunctionType.Exp,
                                 scale=NEG_SLOPE)
            nc.scalar.activation(Lt[:, 0:RLs, :], Lt[:, 0:RLs, :],
                                 mybir.ActivationFunctionType.Exp)
            nc.vector.tensor_tensor(Gt[:, 0:RLs, F_H:F_H + F_D],
                                    Lt[:, 0:RLs, :], E2t[:, 0:RLs, :],
                                    op=mybir.AluOpType.max)
            # msg = w * h, one broadcast-view multiply for all heads
            nc.vector.tensor_mul(
                Gt[:, 0:RLs, 0:F_H].rearrange("p s (h c) -> p s h c", c=CW),
                Gt[:, 0:RLs, 0:F_H].rearrange("p s (h c) -> p s h c", c=CW),
                Gt[:, 0:RLs, F_H:F_H + F_D].unsqueeze(3).to_broadcast(
                    (P, RLs, F_D, CW)))

            # aggregation: L-slots folded into the accumulation group
            red = opool.tile([P, GSZ, F_G], F32)
            for j in range(gn):
                t = t0 + j
                chunks = tile_chunks[t]
                psE = ppe.tile([P, F_G], F32)
                nmm = len(chunks) * L
                mi = 0
                for ci, cid in enumerate(chunks):
                    rc = cid - gbase
                    for l in range(L):
                        nc.tensor.matmul(
                            psE[:], St[:, rc, :],
                            Gt[:, rc * L + l, 0:F_G],
                            start=(mi == 0), stop=(mi == nmm - 1))
                        mi += 1
                nc.scalar.activation(red[:, j, :], psE[:],
                                     mybir.ActivationFunctionType.Copy)

            # ---- postprocess ----
            den = opool.tile([P, GSZ, F_D], F32)
            nc.vector.tensor_scalar_add(den[:, 0:gn, :],
                                        red[:, 0:gn, F_H:F_G], eps)
            rec = opool.tile([P, GSZ, F_D], F32)
            nc.vector.reciprocal(rec[:, 0:gn, :], den[:, 0:gn, :])
            o1 = opool.tile([P, GSZ, F_H], F32)
            nc.vector.tensor_mul(
                o1[:, 0:gn, :].rearrange("p g (h c) -> p g h c", c=CW),
                red[:, 0:gn, 0:F_H].rearrange(
                    "p g (h c) -> p g h c", c=CW),
                rec[:, 0:gn, :].unsqueeze(3).to_broadcast(
                    (P, gn, F_D, CW)))
            nc.vector.tensor_add(o1[:, 0:gn, :], o1[:, 0:gn, :],
                                 bias_sb[:].unsqueeze(1).to_broadcast(
                                     (P, gn, F_H)))
            if layer == 1:
                mn = opool.tile([P, GSZ, F_H], F32)
                nc.vector.tensor_scalar_min(mn[:, 0:gn, :], o1[:, 0:gn, :],
                                            0.0)
                em = opool.tile([P, GSZ, F_H], F32)
                nc.scalar.activation(em[:, 0:gn, :], mn[:, 0:gn, :],
                                     mybir.ActivationFunctionType.Exp)
                mx = opool.tile([P, GSZ, F_H], F32)
                nc.vector.tensor_scalar_max(mx[:, 0:gn, :], o1[:, 0:gn, :],
                                            0.0)
                s1 = opool.tile([P, GSZ, F_H], F32)
                nc.vector.tensor_add(s1[:, 0:gn, :], mx[:, 0:gn, :],
                                     em[:, 0:gn, :])
                ob = opool.tile([P, GSZ, F_H], BF16)
                nc.vector.tensor_scalar_add(ob[:, 0:gn, :], s1[:, 0:gn, :],
                                            -1.0)
                nc.scalar.dma_start(
                    out_dram[ds(t0 * P, gn * P), :].rearrange(
                        "(j p) c -> p j c", p=P),
                    ob[:, 0:gn, :])
            else:
                rm = opool.tile([P, GSZ, 1], F32)
                nc.vector.tensor_reduce(rm[:, 0:gn, :], o1[:, 0:gn, :],
                                        mybir.AxisListType.X,
                                        mybir.AluOpType.max)
                nc.vector.tensor_tensor(
                    zbuf[:, t0:t0 + gn, :], o1[:, 0:gn, :],
                    rm[:, 0:gn, :].to_broadcast((P, gn, F_H)),
                    op=mybir.AluOpType.subtract)
                for j in range(gn):
                    nc.scalar.activation(
                        o1[:, j, :], zbuf[:, t0 + j, :],
                        mybir.ActivationFunctionType.Exp,
                        accum_out=sums[:, t0 + j:t0 + j + 1])
                ls = opool.tile([P, GSZ], F32)
                nc.scalar.activation(ls[:, 0:gn], sums[:, t0:t0 + gn],
                                     mybir.ActivationFunctionType.Ln)
                of = opool.tile([P, GSZ, F_H], F32)
                nc.vector.tensor_tensor(
                    of[:, 0:gn, :], zbuf[:, t0:t0 + gn, :],
                    ls[:, 0:gn].unsqueeze(2).to_broadcast((P, gn, F_H)),
                    op=mybir.AluOpType.subtract)
                nc.scalar.dma_start(
                    out_dram[ds(t0 * P, gn * P), :].rearrange(
                        "(j p) c -> p j c", p=P),
                    of[:, 0:gn, :])

    nc.compile()
    return nc


def _fold_weights1(W1, att_src1, att_dst1):
    A1s = np.zeros((64, 8), np.float32)
    A1s[np.arange(64), np.arange(64) // 8] = np.asarray(
        att_src1, np.float32).reshape(64)
    A1d = np.zeros((64, 8), np.float32)
    A1d[np.arange(64), np.arange(64) // 8] = np.asarray(
        att_dst1, np.float32).reshape(64)
    wc = np.zeros((256, TW), np.float32)
    wc[:, 0:64] = W1
    wc[:, 64:72] = W1 @ A1s
    wc[:, 72:80] = W1 @ A1d
    return wc


def kernel(x, edge_index, W1, att_src1, att_dst1, bias1,
           W2, att_src2, att_dst2, bias2):
    LAST_RESULTS.clear()
    meta = _prep_edges(edge_index)
    perms = meta["perms"]
    pois = np.full((1, TW - 64), POISON_VAL, NPBF16)

    def in_maps_for(xT_list, wc, bias):
        return [{
            "xT": xT_list[k], "wc": wc,
            "idx16": np.ascontiguousarray(meta["idx16"][k]),
            "rdf": np.ascontiguousarray(meta["rdf"][k]),
            "rdfR": np.ascontiguousarray(np.broadcast_to(
                meta["rdfR"][k].reshape(1, -1), (P, meta["rdfR"][k].size))),
            "bias": np.asarray(bias, np.float32).reshape(1, 64),
            "pois": pois,
        } for k in range(NCORES)]

    def make_xT(xf):
        """xf [N, K] f32/bf16 -> per-core permuted xT [K, NPAD] bf16."""
        K = xf.shape[1]
        out = []
        xTb = np.ascontiguousarray(xf.T.astype(NPBF16))  # [K, N]
        for k in range(NCORES):
            xk = np.zeros((K, NPAD), NPBF16)
            xk[:, perms[k]] = xTb
            out.append(xk)
        return out

    # ---------- layer 1 ----------
    Wc1 = _fold_weights1(W1, att_src1, att_dst1).astype(NPBF16)
    xT_l = make_xT(np.asarray(x, np.float32))
    nc1 = _build_layer_program(256, 8, meta, layer=1)
    res1 = run_bass_kernel_spmd(nc1, in_maps_for(xT_l, Wc1, bias1),
                                core_ids=list(range(NCORES)))
    LAST_RESULTS.append(res1)
    # rows 0..NREAL-1 of core k = nodes k*NREAL..(k+1)*NREAL-1
    x2 = np.concatenate([res1.results[k]["out"][:NREAL]
                         for k in range(NCORES)], axis=0)  # [N, 64] bf16

    # ---------- layer 2 ----------
    Wc2 = np.zeros((64, TW), np.float32)
    Wc2[:, 0:64] = W2
    Wc2[:, 64:65] = W2 @ np.asarray(att_src2, np.float32).T
    Wc2[:, 65:66] = W2 @ np.asarray(att_dst2, np.float32).T
    Wc2 = Wc2.astype(NPBF16)
    xT2_l = make_xT(x2.astype(np.float32))
    nc2 = _build_layer_program(64, 1, meta, layer=2)
    res2 = run_bass_kernel_spmd(nc2, in_maps_for(xT2_l, Wc2, bias2),
                                core_ids=list(range(NCORES)))
    LAST_RESULTS.append(res2)
    out = np.concatenate([res2.results[k]["out"][:NREAL]
                          for k in range(NCORES)], axis=0)
    return out[:N].astype(np.float32)
